# revision 9
# baseline (speedup 1.0000x reference)
"""Trainium2 kernel for nn_MultiHeadCrossAttention_28063316313030.

Math: with seq_len == 1, softmax over a size-1 axis is identically 1, so
attention(Q,K,V) == V and W_Q/W_K are dead code.  The whole module collapses to

    out = LN(x1 @ A) + LN(x2 @ A),   A = W_V.T @ W_fc.T   (1024 x 1024)

where LN is LayerNorm over the last dim with gamma/beta.  The host centers
A's rows over the output dim (A' = A - mean_o A), which makes mean_o(x @ A')
exactly zero, so LN reduces to z * rsqrt(mean(z^2) + eps) on device.

Distribution: pure data parallel over the batch dim across 8 NeuronCores.

Device per core (2048 rows per stream, fp16 matmuls, fp16 output):
  PE warmup matmuls source a memset SBUF tile (no DMA dependency), so the
  PE starts within the framework preamble and holds the HAM clock gate open
  until real data lands.  DMA triggers cost ~610ns each on the Sync engine,
  so inputs are batched into few large descriptors: x bt0 first (both
  streams packed in one host array), then the 8 per-k A chunks, then the
  remaining x row-tiles in growing batches.  b-tiles are processed in
  PAIRS (both streams), k-major into 8 PSUM banks.
  Epilogue per group: bn_stats/bn_aggr -> var (DVE), sqrt+eps (ACT),
  reciprocal (DVE), n = z*r (ACT Identity, PSUM->SBUF, frees the banks).
  out = n1 + n2 on GpSimd (fp16), one DMA per row tile; the last tile
  stores per 512-half so the final DMA waits only on the last half.
"""

import sys

sys.path.insert(0, "/opt/trn_rl_repo")

import numpy as np

B, C, OUT = 16384, 1024, 1024
EPS = 1e-5
NCORES = 8
R = B // NCORES  # rows per core per stream
P = 128
KT = C // P  # contraction tiles
BT = R // P  # row tiles per core
NH = OUT // 512  # psum bank halves per row tile
N_WARMUP = 16
WARM_N = 256
# x DMA batches: row-tile ranges, first small for fast start
X_BATCHES = [(0, 1), (1, 2), (2, 4), (4, 7), (7, 11), (11, 16)]

_cache = {}


def _build(use_affine: bool, mm_dtype_name: str):
    import concourse.bacc as bacc
    import concourse.bass as bass
    import concourse.mybir as mybir
    from concourse.tile import TileContext

    f32 = mybir.dt.float32
    f16 = mybir.dt.float16
    mmdt = getattr(mybir.dt, mm_dtype_name)
    AF = mybir.ActivationFunctionType
    ALU = mybir.AluOpType

    nc = bacc.Bacc("TRN2", target_bir_lowering=False, debug=False, num_devices=NCORES)

    # host-packed: [ki, bt, (s, ko, bi) flattened to 2048]
    x_d = nc.declare_dram_parameter("xall", [P, BT, 2 * KT * P], mmdt, isOutput=False)
    # host-pretiled: [ki, (ko, o) flattened to KT*OUT]
    a_d = nc.declare_dram_parameter("a", [P, KT * OUT], mmdt, isOutput=False)
    if use_affine:
        gam_d = nc.declare_dram_parameter("gamma", [OUT], f32, isOutput=False)
        bet2_d = nc.declare_dram_parameter("beta2", [OUT], f32, isOutput=False)
    y_d = nc.declare_dram_parameter("y", [R, OUT], f16, isOutput=True)

    with TileContext(nc) as tc:
        with (
            tc.tile_pool(name="singles", bufs=1) as singles,
            tc.tile_pool(name="ns", bufs=3) as npool,
            tc.tile_pool(name="outs", bufs=3) as opool,
            tc.tile_pool(name="stats", bufs=4) as stats,
            tc.tile_pool(name="psum", bufs=2, space="PSUM") as psum,
        ):
            def psum_tag(s, h):
                return f"ps{s}{h}"

            # --- PE warmup with zero DMA dependency: stationary + moving
            # both come from a memset tile, so the PE begins during the
            # framework preamble and the HAM clock gate is open before the
            # first real matmul.
            # memset on GpSimd: its queue is free ~0.7us before Vector's
            # during the framework preamble, so the PE starts earlier.
            warm_sb = singles.tile([P, 512], mmdt)
            nc.gpsimd.memset(warm_sb, 0.5)
            warm_ps = psum.tile([P, 512], f32, tag=psum_tag(1, 1))
            for w in range(N_WARMUP):
                lo = 128 * (w % 2)
                nc.tensor.matmul(
                    warm_ps[:, 0:WARM_N], lhsT=warm_sb[:, lo : lo + P],
                    rhs=warm_sb[:, 0:WARM_N], start=True, stop=True,
                )

            eps_sb = singles.tile([P, 1], f32)
            nc.gpsimd.memset(eps_sb, EPS)

            # --- input DMAs, few and large; issue order = Sync order.
            # x bt0 first (unblocks the first matmul group), then all of A
            # k-major (k chunks are consumed one per ~0.85us), then the
            # remaining x row-tiles in growing batches.
            xb = {}
            bat0 = X_BATCHES[0]
            t0 = singles.tile(
                [P, bat0[1] - bat0[0], 2 * KT * P], mmdt, name="xb0"
            )
            nc.sync.dma_start(t0[:], x_d[:, bat0[0] : bat0[1]])
            xb[0] = t0

            a_sb = []
            for k in range(KT):
                t = singles.tile([P, OUT], mmdt, name=f"a{k}")
                nc.sync.dma_start(t[:], a_d[:, k * OUT : (k + 1) * OUT])
                a_sb.append(t)

            for bi, (b0, b1) in enumerate(X_BATCHES[1:], start=1):
                t = singles.tile([P, b1 - b0, 2 * KT * P], mmdt, name=f"xb{bi}")
                nc.sync.dma_start(t[:], x_d[:, b0:b1])
                xb[bi] = t

            def xsl(bt, s, k):
                """lhsT AP for (row-tile bt, stream s, k-chunk)."""
                for bi, (b0, b1) in enumerate(X_BATCHES):
                    if b0 <= bt < b1:
                        off = s * KT * P + k * P
                        return xb[bi][:, bt - b0, off : off + P]
                raise AssertionError(bt)

            if use_affine:
                gam_sb = singles.tile([P, OUT], f32)
                nc.sync.dma_start(
                    gam_sb[:],
                    bass.AP(
                        tensor=gam_d.tensor,
                        offset=gam_d.offset,
                        ap=[[0, P], gam_d.ap[0]],
                    ),
                )
                bet2_sb = singles.tile([P, OUT], f32)
                nc.sync.dma_start(
                    bet2_sb[:],
                    bass.AP(
                        tensor=bet2_d.tensor,
                        offset=bet2_d.offset,
                        ap=[[0, P], bet2_d.ap[0]],
                    ),
                )

            def phase1(bt, s, ps_tiles):
                """Stats chain for group (bt, s): psum -> r = 1/sqrt(var+eps)."""
                st = stats.tile([P, NH, 6], f32, tag=f"st{s}", name=f"st{bt}{s}")
                for h in range(NH):
                    nc.vector.bn_stats(st[:, h, :], ps_tiles[h][:])
                mv = stats.tile([P, 2], f32, tag=f"mv{s}", name=f"mv{bt}{s}")
                nc.vector.bn_aggr(mv[:], st[:])
                r_sb = stats.tile([P, 1], f32, tag=f"r{s}", name=f"r{bt}{s}")
                nc.scalar.activation(
                    r_sb[:], mv[:, 1:2], func=AF.Sqrt, bias=eps_sb[:], scale=1.0
                )
                nc.vector.reciprocal(r_sb[:], r_sb[:])
                return r_sb

            def phase2(bt, s, ps_tiles, r_sb):
                """n = z * r (A is host-centered, so the mean term is zero)."""
                ntile = npool.tile([P, OUT], f32, tag=f"n{s}", name=f"n{bt}{s}")
                for h in range(NH):
                    nc.scalar.activation(
                        ntile[:, h * 512 : (h + 1) * 512],
                        ps_tiles[h][:],
                        func=AF.Identity,
                        bias=0.0,
                        scale=r_sb[:],
                    )
                return ntile

            def store(bt, n_pair, split_dma):
                # mid-kernel adds ride the idle GpSimd engine; the last
                # tile's adds use the (by then idle) faster Vector engine so
                # the exposed tail chain is short.
                eng = nc.vector if split_dma else nc.gpsimd
                out_t = opool.tile([P, OUT], f16, tag="out", name=f"out{bt}")
                for h in range(NH):
                    sl = slice(h * 512, (h + 1) * 512)
                    eng.tensor_tensor(
                        out_t[:, sl], n_pair[0][:, sl], n_pair[1][:, sl],
                        op=ALU.add,
                    )
                    if use_affine:
                        eng.tensor_tensor(
                            out_t[:, sl], out_t[:, sl], gam_sb[:, sl], op=ALU.mult
                        )
                        eng.tensor_tensor(
                            out_t[:, sl], out_t[:, sl], bet2_sb[:, sl], op=ALU.add
                        )
                    if split_dma:
                        nc.sync.dma_start(
                            y_d[bt * P : (bt + 1) * P, sl], out_t[:, sl]
                        )
                if not split_dma:
                    nc.sync.dma_start(y_d[bt * P : (bt + 1) * P, :], out_t[:])

            for bt in range(BT):
                ps = {
                    s: [
                        psum.tile(
                            [P, 512], f32, tag=psum_tag(s, h),
                            name=f"ps{bt}{s}{h}",
                        )
                        for h in range(NH)
                    ]
                    for s in range(2)
                }

                last = bt == BT - 1
                if not last:
                    # k-major across both streams: the 4 matmuls per k-chunk
                    # keep the PE slightly slower than the A DMA stream at
                    # kernel start.
                    for k in range(KT):
                        for s in range(2):
                            for h in range(NH):
                                nc.tensor.matmul(
                                    ps[s][h][:],
                                    lhsT=xsl(bt, s, k),
                                    rhs=a_sb[k][:, h * 512 : (h + 1) * 512],
                                    start=(k == 0),
                                    stop=(k == KT - 1),
                                )
                else:
                    # Tail: serialize the streams; the very last stream runs
                    # h-outer so its h0 stats overlap its h1 matmuls.
                    for s in range(2):
                        order = (
                            [(h, k) for h in range(NH) for k in range(KT)]
                            if s == 1
                            else [(h, k) for k in range(KT) for h in range(NH)]
                        )
                        for h, k in order:
                            nc.tensor.matmul(
                                ps[s][h][:],
                                lhsT=xsl(bt, s, k),
                                rhs=a_sb[k][:, h * 512 : (h + 1) * 512],
                                start=(k == 0),
                                stop=(k == KT - 1),
                            )

                # phase1(s) then phase2(s) in stream order keeps the Scalar
                # queue free of head-of-line blocking: s0's identities run
                # during s1's matmuls, so s1's sqrt issues immediately.
                r0 = phase1(bt, 0, ps[0])
                n0 = phase2(bt, 0, ps[0], r0)
                r1 = phase1(bt, 1, ps[1])
                n1 = phase2(bt, 1, ps[1], r1)
                store(bt, [n0, n1], split_dma=last)

    nc.finalize()
    return nc


def _get_nc(use_affine: bool, mm_dtype_name: str):
    key = (use_affine, mm_dtype_name)
    if key not in _cache:
        _cache[key] = _build(use_affine, mm_dtype_name)
    return _cache[key]


def _pretile_x(x_core: np.ndarray) -> np.ndarray:
    # [R, C] -> [ki, bt, ko, bi]
    return np.ascontiguousarray(
        x_core.reshape(BT, P, KT, P).transpose(3, 0, 2, 1)
    )


def kernel(x1, x2, W_Q, W_K, W_V, W_fc, gamma, beta, _trace=False,
           _mm_dtype="float16"):
    from concourse.bass_utils import run_bass_kernel_spmd

    x1 = np.asarray(x1, dtype=np.float32)
    x2 = np.asarray(x2, dtype=np.float32)
    W_V = np.asarray(W_V, dtype=np.float32)
    W_fc = np.asarray(W_fc, dtype=np.float32)
    gamma = np.asarray(gamma, dtype=np.float32)
    beta = np.asarray(beta, dtype=np.float32)

    # A = W_V.T @ W_fc.T in float64; center rows over the output dim so
    # mean_o(x @ A) == 0 and the device LayerNorm can skip the mean term.
    A = W_V.T.astype(np.float64) @ W_fc.T.astype(np.float64)
    A = (A - A.mean(axis=1, keepdims=True)).astype(np.float32)
    # [C, OUT] -> [ki, ko*o]
    Ap = np.ascontiguousarray(
        A.reshape(KT, P, OUT).transpose(1, 0, 2).reshape(P, KT * OUT)
    )

    use_affine = not (np.all(gamma == 1.0) and np.all(beta == 0.0))

    if _mm_dtype == "bfloat16":
        import ml_dtypes

        np_mm = ml_dtypes.bfloat16
    elif _mm_dtype == "float16":
        np_mm = np.float16
    else:
        np_mm = np.float32
    Ap = Ap.astype(np_mm)

    in_maps = []
    for r in range(NCORES):
        sl = slice(r * R, (r + 1) * R)
        # [ki, bt, s, ko, bi] -> [ki, bt, 2048]
        xall = np.stack(
            [_pretile_x(x1[sl]), _pretile_x(x2[sl])], axis=2
        ).reshape(P, BT, 2 * KT * P)
        m = {
            "xall": np.ascontiguousarray(xall).astype(np_mm),
            "a": Ap,
        }
        if use_affine:
            m["gamma"] = gamma
            m["beta2"] = (2.0 * beta).astype(np.float32)
        in_maps.append(m)

    nc = _get_nc(use_affine, _mm_dtype)
    res = run_bass_kernel_spmd(nc, in_maps, list(range(NCORES)), trace=_trace)

    y = np.concatenate(
        [np.asarray(res.results[r]["y"]) for r in range(NCORES)], axis=0
    ).astype(np.float32)
    out = y.reshape(B, 1, OUT)
    if _trace:
        return out, res
    return out
